# revision 49
# baseline (speedup 1.0000x reference)
"""Multi-head attention (B=2, S=2048, D=2048, H=16 causal) on 8 TRN2 cores.

Sharding: core c handles batch b = c//4 and head group g = c%4 (4 heads,
512 of the 2048 model dims). Tensor-parallel: q/k/v_proj rows (output
dims) are split by head group; o_proj columns (input dims) likewise, so
each core produces a partial [S, D] output that the host sums per batch.

Host prep per core (numpy):
  xt  = x[b].T              [D, S]   bf16   (d on partitions for matmul)
  wqt = q_proj[gslice].T    [D, 512] bf16
  wkt = k_proj[gslice].T    [D, 512] bf16
  wvt = v_proj[gslice].T    [D, 512] bf16
  wot = o_proj[:, gslice].T [512, D] bf16
Device schedule (all matmuls bf16 with f32 PSUM accumulation):
  1. QT/KT [128, 4h, S] projections (chunk-paced against the xt DMA
     stream; a few garbage warmup matmuls flip the HAM clock gate while
     the first operands stream in), then V [128, 16st, 512].
  2. Attention in qb-major rounds (qb = 512-wide query block, so causal
     work grows per round). Per group (h, qb): scoresT[k, q] = KT.T @ QT
     into [128,1024] PSUM pairs, one wide exp on ACT (no max-subtraction:
     |scores| <= ~10), multiplicative causal masks on DVE, pair/quad tree
     sums on DVE feeding [1,512] softmax-denominator matmuls, attnout.T =
     sum_k V[k, dv] * expT[k, q], normalize via reciprocal * broadcast.
     Round 0's scores are interleaved into the V projection stream so the
     ACT exp pipeline is primed before the PE reaches the attention.
  3. o_proj is FUSED into the attention rounds: once round r's four heads
     are normalized, the 16 output chains for s-rows 512r..512r+511
     (out_partial[s, :] = sum_h aoT[:, h, s-chunk].T @ wot[h]) interleave
     into the following rounds' score streams, filling PE stalls that the
     exp pipeline would otherwise leave. Only round 3's chains trail the
     attention, in a 4-bank burst.
"""

import math
import sys
import types

import numpy as np
import ml_dtypes

# If BASS_TRACE is set in the environment, run_bass_kernel_spmd imports
# antenv.axon_hooks, which not every image ships. Register a no-op stub so
# that path degrades to "hook isn't registered" instead of crashing.
try:
    import antenv.axon_hooks  # noqa: F401
except Exception:
    try:
        import antenv

        _stub = types.ModuleType("antenv.axon_hooks")
        _stub._hook = None
        _stub.set_axon_ntff_profile_hook = lambda h: setattr(_stub, "_hook", h)
        _stub.get_axon_ntff_profile_hook = lambda: _stub._hook
        sys.modules["antenv.axon_hooks"] = _stub
        antenv.axon_hooks = _stub
    except Exception:
        pass

import concourse.bass as bass
import concourse.tile as tile
import concourse.mybir as mybir
from concourse import library_config
from concourse.bass_utils import run_bass_kernel_spmd
from concourse.library_overlay import lower_extended_insts
from concourse.vector_clock import ScopedClock

D = 2048
S = 2048
GM = 512  # model dims per core (4 heads x 128)
NH = 4  # heads per core
DK = 128
DC = D // 128  # 16 contraction chunks
NQB = S // 512  # 4 q blocks
SCALE = 1.0 / math.sqrt(DK)
N_CORES = 8

BF16 = mybir.dt.bfloat16
F32 = mybir.dt.float32


def _patched_drain_and_barrier(self, tick_clock, wait_clock):
    # Walrus rejects a Drain carrying >2 sync waits ("Too many sync wait
    # commands"). Put the global-clock waits on standalone single-wait
    # EventSemaphore instructions ahead of the drain instead.
    nc = self.nc
    probe = nc.sync.nop(nofuse=True)
    wait_clock.add_sem_waits(probe.ins, ScopedClock({None: tick_clock.global_clock}))
    si = probe.ins.sync_info
    waits = list(si.on_wait) if si is not None else []
    if len(waits) > 1:
        probe.ins.sync_info = mybir.SyncInfo(
            on_wait=[waits[0]], on_update=list(si.on_update)
        )
        sems = {}
        for h in self.sems.allocated().values():
            sems[h.name] = h
            sems[h.num] = h
        for w in waits[1:]:
            assert w.wait_mode == "sem-ge-imm", w
            h = sems.get(w.ant_name) or sems.get(w.id)
            nc.sync.wait_ge(h, w.wait_value)
    nc.sync.drain()
    nc.all_engine_barrier()
    popped = nc._tile_sem_poison_stack.pop()
    assert popped is self._sem_poison
    nc.clear_and_free_semaphores(list(self.sems.allocated().values()))
    nc.all_engine_barrier()


tile.TileContext._drain_and_barrier = _patched_drain_and_barrier

def _dedup_ldweights(nc):
    """Drop an InstLdweights whose weights AP is identical to the previous
    one on the same basic block with only Matmult/EventSemaphore between —
    the stationary operand is still resident in the PE array, so the reload
    is pure overhead."""
    keep_types = {"InstMatmult", "InstEventSemaphore"}
    n_drop = 0
    for fn in nc.m.functions:
        for bb in fn.blocks:
            out = []
            last_key = None
            for inst in bb.instructions:
                tname = type(inst).__name__
                if tname == "InstLdweights":
                    si = inst.sync_info
                    key = (str(inst.ins[0]), getattr(inst, "tile_position", None))
                    if last_key == key and not (si and si.on_update):
                        if si and si.on_wait:
                            ev = mybir.InstEventSemaphore(
                                name=nc.get_next_instruction_name(),
                                engine=inst.engine,
                                ins=[],
                                outs=[],
                                sync_info=mybir.SyncInfo(
                                    on_wait=list(si.on_wait), on_update=[]
                                ),
                            )
                            nc.register_instruction(ev)
                            out.append(ev)
                        del nc.inst_map[inst.name]
                        n_drop += 1
                        continue
                    last_key = key
                elif tname not in keep_types and str(inst.engine) == "EngineType.PE":
                    last_key = None
                out.append(inst)
            bb.instructions[:] = out
    return n_drop


def _split_excess_waits(nc, max_waits=1):
    """Walrus rejects instructions carrying more than a couple of sync wait
    commands. Move excess waits onto standalone EventSemaphore instructions
    inserted just before the offender on the same engine (same-queue program
    order makes this equivalent)."""
    for fn in nc.m.functions:
        for bb in fn.blocks:
            out = []
            for inst in bb.instructions:
                si = inst.sync_info
                if si is not None and len(si.on_wait) > max_waits:
                    waits = list(si.on_wait)
                    for w in waits[:-max_waits]:
                        ev = mybir.InstEventSemaphore(
                            name=nc.get_next_instruction_name(),
                            engine=inst.engine,
                            ins=[],
                            outs=[],
                            sync_info=mybir.SyncInfo(on_wait=[w], on_update=[]),
                        )
                        nc.register_instruction(ev)
                        out.append(ev)
                    inst.sync_info = mybir.SyncInfo(
                        on_wait=waits[-max_waits:], on_update=list(si.on_update)
                    )
                out.append(inst)
            bb.instructions[:] = out


def build_bass():
    nc = bass.Bass("TRN2", target_bir_lowering=False, debug=False, num_devices=N_CORES)

    xt_d = nc.declare_dram_parameter("xt", [D, S], BF16, isOutput=False)
    wqt_d = nc.declare_dram_parameter("wqt", [D, GM], BF16, isOutput=False)
    wkt_d = nc.declare_dram_parameter("wkt", [D, GM], BF16, isOutput=False)
    wvt_d = nc.declare_dram_parameter("wvt", [D, GM], BF16, isOutput=False)
    wot_d = nc.declare_dram_parameter("wot", [GM, D], BF16, isOutput=False)
    masks_d = nc.declare_dram_parameter("masks", [128, NQB * 512], BF16, isOutput=False)
    ones_d = nc.declare_dram_parameter("ones", [128, 1], BF16, isOutput=False)
    out_d = nc.declare_dram_parameter("out", [S, D], F32, isOutput=True)

    with tile.TileContext(nc) as tc:
        with (
            tc.tile_pool(name="const", bufs=1) as const_pool,
            tc.tile_pool(name="qkv", bufs=1) as qkv_pool,
            tc.tile_pool(name="ao", bufs=1) as ao_pool,
            tc.tile_pool(name="exp0", bufs=4) as exp0_pool,
            tc.tile_pool(name="dp0", bufs=4) as dp0_pool,
        ):
            # GPSIMD runs partition_broadcast; the proxy library has it.
            nc.gpsimd.load_library(library_config.proxy)

            qt_sb = qkv_pool.tile([128, NH, S], BF16)
            kt_sb = qkv_pool.tile([128, NH, S], BF16)
            v_sb = qkv_pool.tile([128, S // 128, GM], BF16)
            ao_sb = ao_pool.tile([128, NH, S], BF16)

            # ---------------- Phase 1: projections ----------------
            # xt / weights live on the RIGHT side of SBUF so releasing them
            # mid-kernel is independent of the long-lived left stack.
            xt_cm = tc.tile_pool(name="xt", bufs=1, side="right")
            xt_pool = xt_cm.__enter__()
            w_cm = tc.tile_pool(name="w", bufs=1, side="right")
            w_pool = w_cm.__enter__()

            xt_sb = xt_pool.tile([128, DC, S], BF16)
            w_tiles = {}
            for wname in ("wq", "wk", "wv"):
                w_tiles[wname] = w_pool.tile(
                    [128, DC, GM], BF16, tag=wname, name=f"w_{wname}"
                )

            def _load_w(wname, wd):
                for dc in range(DC):
                    nc.sync.dma_start(
                        w_tiles[wname][:, dc, :], wd[128 * dc : 128 * (dc + 1), :]
                    )

            # Warmup source: the HAM clock gate keeps the PE at 1.2 GHz
            # until it has been ~3.4us busy. Garbage matmuls with no DMA
            # dependency flip it to 2.4 GHz while real operands stream in.
            # A raw (non-pool) SBUF tensor skips the tile framework's
            # written-before-read check, so no memset gates the first LDW.
            dummy_cm = nc.sbuf_tensor("warmup_src", [128, 512], BF16)
            dummy_sb = dummy_cm.__enter__()
            # DMA order matters: wq first, then xt chunk-by-chunk so the
            # first QT chain starts early and paces with chunk arrival
            # (each stall < HAM's 3.4us window), then wk/wv.
            # Pass 1 (wq-mtp0) is the only DMA-paced pass: it needs xt
            # plus just the FIRST half of each wq chunk (heads 0-1 =
            # columns 0:256). Defer the second halves to after the xt
            # stream so the bandwidth-limited window carries 9MB, not
            # 10MB, and chunk 1 issues two slots earlier.
            for dc in range(DC):
                if dc == 0:
                    # fine-grained first chunk: the very first matmul
                    # needs only wq[0][:, 0:128] + xt[0][:, 0:512]
                    nc.sync.dma_start(
                        w_tiles["wq"][:, 0, 0:128], wqt_d[0:128, 0:128]
                    )
                    nc.sync.dma_start(xt_sb[:, 0, 0:1024], xt_d[0:128, 0:1024])
                    nc.sync.dma_start(
                        w_tiles["wq"][:, 0, 128:256], wqt_d[0:128, 128:256]
                    )
                    nc.sync.dma_start(
                        xt_sb[:, 0, 1024:2048], xt_d[0:128, 1024:2048]
                    )
                    continue
                nc.sync.dma_start(
                    xt_sb[:, dc, :], xt_d[128 * dc : 128 * (dc + 1), :]
                )
                nc.sync.dma_start(
                    w_tiles["wq"][:, dc, 0:256], wqt_d[128 * dc : 128 * (dc + 1), 0:256]
                )
            for dc in range(DC):
                nc.sync.dma_start(
                    w_tiles["wq"][:, dc, 256:512],
                    wqt_d[128 * dc : 128 * (dc + 1), 256:512],
                )
            _load_w("wk", wkt_d)
            _load_w("wv", wvt_d)
            # Constants from host (needed only for attention — emitted after
            # the projection-critical DMAs): all-ones column for the
            # denominator matmuls, and 4 shifted-triangular causal masks
            # (variant r keeps q >= k + 128r).
            ones_sb = const_pool.tile([128, 1], BF16)
            nc.sync.dma_start(ones_sb[:], ones_d[:])
            masks_sb = const_pool.tile([128, NQB, 512], BF16)
            nc.sync.dma_start(
                masks_sb[:], masks_d[:].rearrange("p (r q) -> p r q", r=NQB)
            )

            # QT / KT: out tile [m=128, s=512], contraction over d.
            # dc outer over PAIRS of m tiles = 8 concurrent psum chains
            # (all 8 banks): per xt chunk the PE has ~1.7us of work, which
            # matches the chunk DMA arrival rate, so the in-order PE queue
            # does not stall during the load ramp. LDWEIGHTS amortized 4x
            # over the st4-minor matmuls.
            with tc.tile_pool(name="psum_qtkt", bufs=8, space="PSUM") as psum1:
                wu_ps = psum1.tile([128, 512], F32, tag="ps1", name="warmup_ps")
                for _ in range(9):
                    nc.tensor.matmul(
                        wu_ps[:], lhsT=dummy_sb[:, 0:128], rhs=dummy_sb[:],
                        start=True, stop=True,
                    )
                for wname, out_sb in (("wq", qt_sb), ("wk", kt_sb)):
                    w_sb = w_tiles[wname]
                    for mtp in range(NH // 2):
                        last = wname == "wk" and mtp == NH // 2 - 1
                        pss = [
                            psum1.tile(
                                [128, 512],
                                F32,
                                tag="ps1",
                                name=f"ps1_{wname}_{mtp}_{i}",
                            )
                            for i in range(8)
                        ]
                        def _copy_out(j, st4):
                            dst = out_sb[
                                :, 2 * mtp + j, 512 * st4 : 512 * (st4 + 1)
                            ]
                            if last and (2 * j + st4) % 2 == 0:
                                # split the final copy burst across ACT and
                                # DVE so the V projection (which reuses
                                # these psum banks) isn't gated on ~5us of
                                # serial DVE casts — that stall crossed the
                                # HAM window and re-throttled the PE clock.
                                nc.scalar.activation(
                                    dst,
                                    pss[4 * j + st4][:],
                                    mybir.ActivationFunctionType.Copy,
                                )
                            else:
                                nc.vector.tensor_copy(dst, pss[4 * j + st4][:])

                        for dc in range(DC):
                            for j in (0, 1):
                                mt = 2 * mtp + j
                                for st4 in range(NQB):
                                    nc.tensor.matmul(
                                        pss[4 * j + st4][:],
                                        lhsT=w_sb[:, dc, 128 * mt : 128 * (mt + 1)],
                                        rhs=xt_sb[
                                            :, dc, 512 * st4 : 512 * (st4 + 1)
                                        ],
                                        start=(dc == 0),
                                        stop=(dc == DC - 1),
                                    )
                                    if last and dc == DC - 1:
                                        # copy each chain right after its
                                        # stop-matmul, under the remaining
                                        # chains' matmul stream
                                        _copy_out(j, st4)
                        if not last:
                            for j in (0, 1):
                                for st4 in range(NQB):
                                    _copy_out(j, st4)

            # -------- attention psum pools (coexist with the V pool) -----
            # left stack: psum_o (banks 0-1), psum_v (2-3, exits after V);
            # right: psum_s (4-7, two [128,1024] tiles). After V exits:
            # psum_d (bank 2) and psum_3 (bank 3) take psum_v's banks.
            psum_o_cm = tc.tile_pool(name="psum_o", bufs=2, space="PSUM")
            psum_o = psum_o_cm.__enter__()
            psum_v_cm = tc.tile_pool(name="psum_v", bufs=2, space="PSUM")
            psum_v = psum_v_cm.__enter__()
            psum_s_cm = tc.tile_pool(name="psum_s", bufs=2, space="PSUM", side="right")
            psum_s = psum_s_cm.__enter__()

            # Pools bound later (phase C); closures below resolve the names
            # at call time.
            psum_d = psum_3 = small_pool = wot_sb = None

            # ---------------- Attention group machinery ----------------
            def make_score_ops(h, qb, exp_pool, dpair_pool, fixed=None):
                # One op per PAIR of k tiles: two scores matmuls into the
                # two banks of a [128, 1024] PSUM tile, one wide exp, then
                # the causal mask multiplies, then a DVE pair-sum that
                # halves the denominator matmul count.
                nkt = 4 * qb + 4  # k tiles with any unmasked element
                if fixed is None:
                    eT = exp_pool.tile(
                        [128, nkt, 512], BF16, tag="eT", name=f"eT_{h}_{qb}"
                    )
                    dp = dpair_pool.tile(
                        [128, nkt // 2, 512], BF16, tag="dpair", name=f"dp_{h}_{qb}"
                    )
                else:
                    eT = exp_pool.tile(
                        [128, fixed, 512], BF16, tag="eT", name=f"eT_{h}_{qb}"
                    )[:, :nkt, :]
                    dp = dpair_pool.tile(
                        [128, fixed // 2, 512], BF16, tag="dpair",
                        name=f"dp_{h}_{qb}",
                    )[:, : nkt // 2, :]

                def score_pair(kt0):
                    # Diagonal k tiles (r = kt - 4qb >= 0) only have valid
                    # scores for q >= 128r in this block: trim the matmul
                    # to that range. The exp still covers the full pair
                    # tile (the trimmed region holds garbage that is never
                    # read: O/D matmuls are trimmed identically, and the
                    # pair-sum strip is zeroed below).
                    ps = psum_s.tile(
                        [128, 1024], F32, tag="ps_s", name=f"ps_s_{h}_{qb}_{kt0}"
                    )
                    for j in (0, 1):
                        kt = kt0 + j
                        r = kt - 4 * qb
                        qo = 128 * r if r > 0 else 0
                        nc.tensor.matmul(
                            ps[:, 512 * j + qo : 512 * (j + 1)],
                            lhsT=kt_sb[:, h, 128 * kt : 128 * (kt + 1)],
                            rhs=qt_sb[:, h, 512 * qb + qo : 512 * (qb + 1)],
                            start=True,
                            stop=True,
                        )
                    r1 = kt0 + 1 - 4 * qb
                    if r1 < 0:
                        # non-diagonal pair: one wide exp over both banks
                        nc.scalar.activation(
                            eT[:, kt0 : kt0 + 2, :].rearrange("p a b -> p (a b)"),
                            ps[:],
                            mybir.ActivationFunctionType.Exp,
                            scale=SCALE,
                        )
                    else:
                        # diagonal pair: exact-range exps (the trimmed
                        # PSUM regions were never written)
                        for j in (0, 1):
                            r = kt0 + j - 4 * qb
                            qo = 128 * r if r > 0 else 0
                            nc.scalar.activation(
                                eT[:, kt0 + j, qo:512],
                                ps[:, 512 * j + qo : 512 * (j + 1)],
                                mybir.ActivationFunctionType.Exp,
                                scale=SCALE,
                            )
                    for j in (0, 1):
                        r = kt0 + j - 4 * qb
                        if r >= 0:
                            # only the 128-wide diagonal strip can have
                            # masked elements; beyond it the mask is all 1
                            qo = 128 * r
                            qe = qo + 128
                            nc.vector.tensor_mul(
                                eT[:, kt0 + j, qo:qe],
                                eT[:, kt0 + j, qo:qe],
                                masks_sb[:, r, qo:qe],
                            )
                    r1 = kt0 + 1 - 4 * qb
                    if r1 >= 1:
                        qo0 = 128 * (r1 - 1)
                        qo1 = 128 * r1
                        # kt0+1's exp starts at qo1; below it only kt0
                        # contributes — copy that strip instead of zeroing
                        # the garbage and paying for a wider add
                        nc.vector.tensor_copy(
                            dp[:, kt0 // 2, qo0:qo1], eT[:, kt0, qo0:qo1]
                        )
                        nc.vector.tensor_add(
                            dp[:, kt0 // 2, qo1:512],
                            eT[:, kt0, qo1:512],
                            eT[:, kt0 + 1, qo1:512],
                        )
                    else:
                        nc.vector.tensor_add(
                            dp[:, kt0 // 2, :], eT[:, kt0, :], eT[:, kt0 + 1, :]
                        )
                        # second-level sum for clean (non-diagonal) quads:
                        # dp[2q] += dp[2q+1] in place, halving the [1,512]
                        # denominator matmuls for the off-diagonal k range
                        p = kt0 // 2
                        if p % 2 == 1 and p <= 2 * qb - 1:
                            nc.vector.tensor_add(
                                dp[:, p - 1, :], dp[:, p - 1, :], dp[:, p, :]
                            )

                return (
                    eT,
                    dp,
                    [lambda k=k: score_pair(2 * k) for k in range(nkt // 2)],
                )

            def make_av_ops(h, qb, eT, dp):
                nkt = 4 * qb + 4
                qsl = slice(512 * qb, 512 * (qb + 1))
                state = {}

                def o_op(kt):
                    if kt == 0:
                        state["psO"] = psum_o.tile(
                            [128, 512], F32, tag="ps_o", name=f"psO_{h}_{qb}"
                        )
                    r = kt - 4 * qb
                    qo = 128 * r if r > 0 else 0
                    nc.tensor.matmul(
                        state["psO"][:, qo:512],
                        lhsT=v_sb[:, kt, 128 * h : 128 * (h + 1)],
                        rhs=eT[:, kt, qo:512],
                        start=(kt == 0),
                        stop=(kt == nkt - 1),
                    )

                clean = [2 * q for q in range(qb)]  # quad-summed reads
                n_d = len(clean) + 2

                def d_op(i):
                    # Over the DVE tree sums; emitted as one consecutive
                    # burst so the identical all-ones LDWEIGHTS dedupe
                    # down to a single load.
                    if i == 0:
                        state["psD"] = psum_d.tile(
                            [1, 512], F32, tag="ps_d", name=f"psD_{h}_{qb}"
                        )
                    if i < len(clean):
                        p, qo = clean[i], 0
                    else:
                        p = 2 * qb + (i - len(clean))  # diagonal pair
                        r0 = 2 * p - 4 * qb
                        qo = 128 * r0 if r0 > 0 else 0
                    nc.tensor.matmul(
                        state["psD"][0:1, qo:512],
                        lhsT=ones_sb[:, :],
                        rhs=dp[:, p, qo:512],
                        start=(i == 0),
                        stop=(i == n_d - 1),
                    )

                def fin_a():
                    # reciprocal_approx_fast (~18 bits; denominators are
                    # well in range) straight from PSUM, broadcast on
                    # GPSIMD. Emitted right after the D chain so the slow
                    # (~1-2us) broadcast overlaps the slot's scores/AV
                    # instead of gating the psum_o recycle at its end.
                    rcp = small_pool.tile(
                        [1, 512], F32, tag="rcp", name=f"rcp_{h}_{qb}"
                    )
                    nc.vector.reciprocal_approx_fast(rcp[:], state["psD"][:])
                    rcpb = small_pool.tile(
                        [128, 512], F32, tag="rcpb", name=f"rcpb_{h}_{qb}"
                    )
                    nc.gpsimd.partition_broadcast(rcpb[:], rcp[:])
                    state["rcpb"] = rcpb

                def fin_b():
                    # normalize on DVE once the AV chain has the psO ready
                    nc.vector.tensor_mul(
                        ao_sb[:, h, qsl], state["psO"][:], state["rcpb"][:]
                    )

                return (
                    [lambda kt=kt: o_op(kt) for kt in range(nkt)],
                    [lambda i=i: d_op(i) for i in range(n_d)],
                    fin_a,
                    fin_b,
                )

            # ----- V projection with round-0 scores interleaved -----
            # V: out tile [s=128, dv=512]; stationary is the xt chunk, so
            # every matmul reloads weights — the 64-deep PE queue pulls the
            # LDWEIGHTS ahead of the running matmul, hiding the reload.
            grp0 = {}
            for st in range(S // 128):
                ps = psum_v.tile([128, 512], F32, tag="psv", name=f"psv_{st}")
                w_sb = w_tiles["wv"]
                for dc in range(DC):
                    nc.tensor.matmul(
                        ps[:],
                        lhsT=xt_sb[:, dc, 128 * st : 128 * (st + 1)],
                        rhs=w_sb[:, dc, :],
                        start=(dc == 0),
                        stop=(dc == DC - 1),
                    )
                nc.vector.tensor_copy(v_sb[:, st, :], ps[:])
                # Prime the attention pipeline: scores for all of round 0
                # under the early V chains. QT/KT are long done; the exps
                # run on ACT underneath the remaining V stream.
                if st in (0, 1, 2, 4):
                    h = 3 if st == 4 else st
                    eT, dp, s_ops = make_score_ops(h, 0, exp0_pool, dp0_pool)
                    for op in s_ops:
                        op()
                    grp0[h] = (eT, dp)

            psum_v_cm.__exit__(None, None, None)
            psum_d_cm = tc.tile_pool(name="psum_d", bufs=1, space="PSUM")
            psum_d = psum_d_cm.__enter__()
            psum_3_cm = tc.tile_pool(name="psum_3", bufs=1, space="PSUM")
            psum_3 = psum_3_cm.__enter__()

            # xt / weights are dead; the attention + o_proj SBUF working
            # set replaces them (right-side frees don't disturb the left
            # stack the new pools go on).
            w_cm.__exit__(None, None, None)
            xt_cm.__exit__(None, None, None)

            exp_cm = tc.tile_pool(name="exp", bufs=2)
            exp_pool = exp_cm.__enter__()
            dpair_cm = tc.tile_pool(name="dpair", bufs=2)
            dpair_pool = dpair_cm.__enter__()
            small_cm = tc.tile_pool(name="small", bufs=3)
            small_pool = small_cm.__enter__()
            wot_cm = tc.tile_pool(name="wot", bufs=1)
            wot_pool = wot_cm.__enter__()
            out_cm = tc.tile_pool(name="ostage", bufs=6)
            out_pool = out_cm.__enter__()

            wot_sb = wot_pool.tile([128, NH, D], BF16)
            for c4 in range(NH):
                nc.sync.dma_start(
                    wot_sb[:, c4, :], wot_d[128 * c4 : 128 * (c4 + 1), :]
                )

            # ---------------- Fused attention + o_proj rounds ----------
            def p3_chain(st, nt):
                ps = psum_3.tile([128, 512], F32, tag="ps3", name=f"ps3_{st}_{nt}")
                for h in range(NH):
                    nc.tensor.matmul(
                        ps[:],
                        lhsT=ao_sb[:, h, 128 * st : 128 * (st + 1)],
                        rhs=wot_sb[:, h, 512 * nt : 512 * (nt + 1)],
                        start=(h == 0),
                        stop=(h == NH - 1),
                    )
                o_sb = out_pool.tile(
                    [128, 512], F32, tag="ost", name=f"ost_{st}_{nt}"
                )
                # copies on DVE: ACT's FIFO must stay clear for the exps
                # that pace the whole attention pipeline
                nc.vector.tensor_copy(o_sb[:], ps[:])
                nc.sync.dma_start(
                    out_d[128 * st : 128 * (st + 1), 512 * nt : 512 * (nt + 1)],
                    o_sb[:],
                )

            pend_p3 = []  # ready (st, nt) o_proj chains, FIFO

            # Two-slot software pipeline: slot i emits the scores of group
            # X[i] and the AV/denominator/normalize of group Y[i] = the
            # group scored two slots earlier. The 2-slot lead means the
            # exp (ACT) of a group is long done before its AV matmuls, so
            # the PE never stalls on the exp pipeline; the eT/dp rings
            # (bufs=2) self-throttle ACT/DVE against the PE's progress.
            # Round 0 was scored under the V stream, so Y starts at round
            # 0 while X starts at round 1.
            records = dict(grp0)  # (h,qb)->(eT,dp) — round 0 keyed by h
            x_seq = (
                [(0, 1), (1, 1), None, None, (2, 1), (3, 1)]
                + [(h, qb) for qb in (2, 3) for h in range(NH)]
                + [None, None]
            )
            y_seq = [(h, qb) for qb in range(NQB) for h in range(NH)]
            for X, Y in zip(x_seq, y_seq):
                yh, yqb = Y
                eT, dp = records.pop(yh if yqb == 0 else Y)
                o_ops, d_ops, fin_a, fin_b = make_av_ops(yh, yqb, eT, dp)
                n_o = len(o_ops)
                emitted = 0
                while emitted < min(2, n_o):
                    o_ops[emitted]()
                    emitted += 1
                # denominator chain + reciprocal/broadcast up front: the
                # dp sums are two slots old, and the broadcast latency
                # hides under this slot's scores/AV stream
                for op in d_ops:
                    op()
                fin_a()
                if X is not None:
                    eTx, dpx, s_ops = make_score_ops(
                        X[0], X[1], exp_pool, dpair_pool, fixed=16
                    )
                    records[X] = (eTx, dpx)
                    n_s = len(s_ops)
                    for i, s in enumerate(s_ops):
                        s()
                        want = (i + 1) * n_o // n_s
                        while emitted < want:
                            o_ops[emitted]()
                            emitted += 1
                        # one fused o_proj chain per score pair keeps
                        # ~0.9us of other PE work between chains (psum_3
                        # is one bank, so back-to-back chains would stall
                        # on the DVE drain).
                        if pend_p3 and i >= 1:
                            p3_chain(*pend_p3.pop(0))
                while emitted < n_o:
                    o_ops[emitted]()
                    emitted += 1
                    if X is None and pend_p3 and emitted % 2 == 0:
                        p3_chain(*pend_p3.pop(0))
                fin_b()
                if yh == NH - 1:
                    # round yqb fully normalized: its 16 o_proj chains are
                    # ready to fuse into the following slots' streams.
                    pend_p3.extend(
                        (4 * yqb + sti, nt)
                        for sti in range(4)
                        for nt in range(NQB)
                    )
            # drain any chains that didn't fit between the last score pairs
            # while the single-bank pool still exists; spacing no longer
            # matters once PSUM copies are the only outstanding PE deps...
            # instead, swap to a 4-bank pool for the trailing burst.
            psum_3_cm.__exit__(None, None, None)
            psum_d_cm.__exit__(None, None, None)
            psum_s_cm.__exit__(None, None, None)

            # ---------------- trailing o_proj chains ----------------
            # Whatever is left (at least round 3): 4 concurrent chains so
            # the psum->SBUF copies overlap the next chains' matmuls.
            with tc.tile_pool(name="psum_3t", bufs=4, space="PSUM") as psum_3t:
                rest = list(pend_p3)
                pend_p3.clear()
                by_st = {}
                for st, nt in rest:
                    by_st.setdefault(st, []).append(nt)
                for st, nts in by_st.items():
                    # nt-outer / h-inner: each chain completes after its 4
                    # matmuls, so its copy (alternating DVE/ACT) overlaps
                    # the remaining chains and the next st's ring reuse
                    # never waits on a copy still in flight.
                    for nt in nts:
                        ps = psum_3t.tile(
                            [128, 512], F32, tag="ps3t", name=f"ps3t_{st}_{nt}"
                        )
                        for h in range(NH):
                            nc.tensor.matmul(
                                ps[:],
                                lhsT=ao_sb[:, h, 128 * st : 128 * (st + 1)],
                                rhs=wot_sb[:, h, 512 * nt : 512 * (nt + 1)],
                                start=(h == 0),
                                stop=(h == NH - 1),
                            )
                        o_sb = out_pool.tile(
                            [128, 512], F32, tag="ost", name=f"ost_{st}_{nt}"
                        )
                        if nt % 2 == 0:
                            nc.scalar.activation(
                                o_sb[:],
                                ps[:],
                                mybir.ActivationFunctionType.Copy,
                            )
                        else:
                            nc.vector.tensor_copy(o_sb[:], ps[:])
                        nc.sync.dma_start(
                            out_d[
                                128 * st : 128 * (st + 1),
                                512 * nt : 512 * (nt + 1),
                            ],
                            o_sb[:],
                        )
            psum_o_cm.__exit__(None, None, None)
            out_cm.__exit__(None, None, None)
            wot_cm.__exit__(None, None, None)
            small_cm.__exit__(None, None, None)
            dpair_cm.__exit__(None, None, None)
            exp_cm.__exit__(None, None, None)
    _dedup_ldweights(nc)
    _split_excess_waits(nc)
    # Populate .instr bytes for extended-inst InstISA subclasses
    # (InstPartitionBroadcast) — raw Bass skips this Bacc pass and the NEFF
    # compiler errors with "ISA wrong length" without it.
    lower_extended_insts(nc)
    return nc


def _prep_in_maps(in_features, q_proj, k_proj, v_proj, o_proj):
    # Host-side prep in numpy — np.asarray first so jax-array inputs don't
    # route the transpose/cast through a device backend.
    in_features = np.asarray(in_features)
    q_proj = np.asarray(q_proj)
    k_proj = np.asarray(k_proj)
    v_proj = np.asarray(v_proj)
    o_proj = np.asarray(o_proj)
    bf = ml_dtypes.bfloat16
    # mask variant r: [128, 512] keeping (1.0) where q >= k + 128r, else 0.
    k_idx = np.arange(128)[:, None]
    q_idx = np.arange(512)[None, :]
    masks = np.concatenate(
        [(q_idx >= k_idx + 128 * r) for r in range(NQB)], axis=1
    ).astype(bf)
    ones = np.ones((128, 1), bf)
    in_maps = []
    for c in range(N_CORES):
        b, g = divmod(c, 4)
        ms = slice(512 * g, 512 * (g + 1))
        in_maps.append(
            {
                "xt": in_features[b].T.astype(bf),
                "wqt": q_proj[ms, :].T.astype(bf),
                "wkt": k_proj[ms, :].T.astype(bf),
                "wvt": v_proj[ms, :].T.astype(bf),
                "wot": o_proj[:, ms].T.astype(bf),
                "masks": masks,
                "ones": ones,
            }
        )
    return in_maps


def _run(inputs, trace=False):
    nc = build_bass()
    in_maps = _prep_in_maps(**inputs)
    res = run_bass_kernel_spmd(nc, in_maps, list(range(N_CORES)), trace=trace)
    B = inputs["in_features"].shape[0]
    out = np.zeros((B, S, D), np.float32)
    for c in range(N_CORES):
        out[c // 4] += res.results[c]["out"]
    return out, res


def kernel(**inputs):
    out, _ = _run(inputs, trace=False)
    return out


# revision 50
# speedup vs baseline: 1.0041x; 1.0041x over previous
"""Multi-head attention (B=2, S=2048, D=2048, H=16 causal) on 8 TRN2 cores.

Sharding: core c handles batch b = c//4 and head group g = c%4 (4 heads,
512 of the 2048 model dims). Tensor-parallel: q/k/v_proj rows (output
dims) are split by head group; o_proj columns (input dims) likewise, so
each core produces a partial [S, D] output that the host sums per batch.

Host prep per core (numpy):
  xt  = x[b].T              [D, S]   bf16   (d on partitions for matmul)
  wqt = q_proj[gslice].T    [D, 512] bf16
  wkt = k_proj[gslice].T    [D, 512] bf16
  wvt = v_proj[gslice].T    [D, 512] bf16
  wot = o_proj[:, gslice].T [512, D] bf16
Device schedule (all matmuls bf16 with f32 PSUM accumulation):
  1. QT/KT [128, 4h, S] projections (chunk-paced against the xt DMA
     stream; a few garbage warmup matmuls flip the HAM clock gate while
     the first operands stream in), then V [128, 16st, 512].
  2. Attention in qb-major rounds (qb = 512-wide query block, so causal
     work grows per round). Per group (h, qb): scoresT[k, q] = KT.T @ QT
     into [128,1024] PSUM pairs, one wide exp on ACT (no max-subtraction:
     |scores| <= ~10), multiplicative causal masks on DVE, pair/quad tree
     sums on DVE feeding [1,512] softmax-denominator matmuls, attnout.T =
     sum_k V[k, dv] * expT[k, q], normalize via reciprocal * broadcast.
     Round 0's scores are interleaved into the V projection stream so the
     ACT exp pipeline is primed before the PE reaches the attention.
  3. o_proj is FUSED into the attention rounds: once round r's four heads
     are normalized, the 16 output chains for s-rows 512r..512r+511
     (out_partial[s, :] = sum_h aoT[:, h, s-chunk].T @ wot[h]) interleave
     into the following rounds' score streams, filling PE stalls that the
     exp pipeline would otherwise leave. Only round 3's chains trail the
     attention, in a 4-bank burst.
"""

import math
import sys
import types

import numpy as np
import ml_dtypes

# If BASS_TRACE is set in the environment, run_bass_kernel_spmd imports
# antenv.axon_hooks, which not every image ships. Register a no-op stub so
# that path degrades to "hook isn't registered" instead of crashing.
try:
    import antenv.axon_hooks  # noqa: F401
except Exception:
    try:
        import antenv

        _stub = types.ModuleType("antenv.axon_hooks")
        _stub._hook = None
        _stub.set_axon_ntff_profile_hook = lambda h: setattr(_stub, "_hook", h)
        _stub.get_axon_ntff_profile_hook = lambda: _stub._hook
        sys.modules["antenv.axon_hooks"] = _stub
        antenv.axon_hooks = _stub
    except Exception:
        pass

import concourse.bass as bass
import concourse.tile as tile
import concourse.mybir as mybir
from concourse import library_config
from concourse.bass_utils import run_bass_kernel_spmd
from concourse.library_overlay import lower_extended_insts
from concourse.vector_clock import ScopedClock

D = 2048
S = 2048
GM = 512  # model dims per core (4 heads x 128)
NH = 4  # heads per core
DK = 128
DC = D // 128  # 16 contraction chunks
NQB = S // 512  # 4 q blocks
SCALE = 1.0 / math.sqrt(DK)
N_CORES = 8

BF16 = mybir.dt.bfloat16
F32 = mybir.dt.float32


def _patched_drain_and_barrier(self, tick_clock, wait_clock):
    # Walrus rejects a Drain carrying >2 sync waits ("Too many sync wait
    # commands"). Put the global-clock waits on standalone single-wait
    # EventSemaphore instructions ahead of the drain instead.
    nc = self.nc
    probe = nc.sync.nop(nofuse=True)
    wait_clock.add_sem_waits(probe.ins, ScopedClock({None: tick_clock.global_clock}))
    si = probe.ins.sync_info
    waits = list(si.on_wait) if si is not None else []
    if len(waits) > 1:
        probe.ins.sync_info = mybir.SyncInfo(
            on_wait=[waits[0]], on_update=list(si.on_update)
        )
        sems = {}
        for h in self.sems.allocated().values():
            sems[h.name] = h
            sems[h.num] = h
        for w in waits[1:]:
            assert w.wait_mode == "sem-ge-imm", w
            h = sems.get(w.ant_name) or sems.get(w.id)
            nc.sync.wait_ge(h, w.wait_value)
    nc.sync.drain()
    nc.all_engine_barrier()
    popped = nc._tile_sem_poison_stack.pop()
    assert popped is self._sem_poison
    nc.clear_and_free_semaphores(list(self.sems.allocated().values()))
    nc.all_engine_barrier()


tile.TileContext._drain_and_barrier = _patched_drain_and_barrier

def _dedup_ldweights(nc):
    """Drop an InstLdweights whose weights AP is identical to the previous
    one on the same basic block with only Matmult/EventSemaphore between —
    the stationary operand is still resident in the PE array, so the reload
    is pure overhead."""
    keep_types = {"InstMatmult", "InstEventSemaphore"}
    n_drop = 0
    for fn in nc.m.functions:
        for bb in fn.blocks:
            out = []
            last_key = None
            for inst in bb.instructions:
                tname = type(inst).__name__
                if tname == "InstLdweights":
                    si = inst.sync_info
                    key = (str(inst.ins[0]), getattr(inst, "tile_position", None))
                    if last_key == key and not (si and si.on_update):
                        if si and si.on_wait:
                            ev = mybir.InstEventSemaphore(
                                name=nc.get_next_instruction_name(),
                                engine=inst.engine,
                                ins=[],
                                outs=[],
                                sync_info=mybir.SyncInfo(
                                    on_wait=list(si.on_wait), on_update=[]
                                ),
                            )
                            nc.register_instruction(ev)
                            out.append(ev)
                        del nc.inst_map[inst.name]
                        n_drop += 1
                        continue
                    last_key = key
                elif tname not in keep_types and str(inst.engine) == "EngineType.PE":
                    last_key = None
                out.append(inst)
            bb.instructions[:] = out
    return n_drop


def _split_excess_waits(nc, max_waits=1):
    """Walrus rejects instructions carrying more than a couple of sync wait
    commands. Move excess waits onto standalone EventSemaphore instructions
    inserted just before the offender on the same engine (same-queue program
    order makes this equivalent)."""
    for fn in nc.m.functions:
        for bb in fn.blocks:
            out = []
            for inst in bb.instructions:
                si = inst.sync_info
                if si is not None and len(si.on_wait) > max_waits:
                    waits = list(si.on_wait)
                    for w in waits[:-max_waits]:
                        ev = mybir.InstEventSemaphore(
                            name=nc.get_next_instruction_name(),
                            engine=inst.engine,
                            ins=[],
                            outs=[],
                            sync_info=mybir.SyncInfo(on_wait=[w], on_update=[]),
                        )
                        nc.register_instruction(ev)
                        out.append(ev)
                    inst.sync_info = mybir.SyncInfo(
                        on_wait=waits[-max_waits:], on_update=list(si.on_update)
                    )
                out.append(inst)
            bb.instructions[:] = out


def build_bass():
    nc = bass.Bass("TRN2", target_bir_lowering=False, debug=False, num_devices=N_CORES)

    xt_d = nc.declare_dram_parameter("xt", [D, S], BF16, isOutput=False)
    wqt_d = nc.declare_dram_parameter("wqt", [D, GM], BF16, isOutput=False)
    wkt_d = nc.declare_dram_parameter("wkt", [D, GM], BF16, isOutput=False)
    wvt_d = nc.declare_dram_parameter("wvt", [D, GM], BF16, isOutput=False)
    wot_d = nc.declare_dram_parameter("wot", [GM, D], BF16, isOutput=False)
    masks_d = nc.declare_dram_parameter("masks", [128, NQB * 512], BF16, isOutput=False)
    ones_d = nc.declare_dram_parameter("ones", [128, 1], BF16, isOutput=False)
    out_d = nc.declare_dram_parameter("out", [S, D], F32, isOutput=True)

    with tile.TileContext(nc) as tc:
        with (
            tc.tile_pool(name="const", bufs=1) as const_pool,
            tc.tile_pool(name="qkv", bufs=1) as qkv_pool,
            tc.tile_pool(name="ao", bufs=1) as ao_pool,
            tc.tile_pool(name="exp0", bufs=4) as exp0_pool,
            tc.tile_pool(name="dp0", bufs=4) as dp0_pool,
        ):
            # GPSIMD runs partition_broadcast; the proxy library has it.
            nc.gpsimd.load_library(library_config.proxy)

            qt_sb = qkv_pool.tile([128, NH, S], BF16)
            kt_sb = qkv_pool.tile([128, NH, S], BF16)
            v_sb = qkv_pool.tile([128, S // 128, GM], BF16)
            ao_sb = ao_pool.tile([128, NH, S], BF16)

            # ---------------- Phase 1: projections ----------------
            # xt / weights live on the RIGHT side of SBUF so releasing them
            # mid-kernel is independent of the long-lived left stack.
            xt_cm = tc.tile_pool(name="xt", bufs=1, side="right")
            xt_pool = xt_cm.__enter__()
            w_cm = tc.tile_pool(name="w", bufs=1, side="right")
            w_pool = w_cm.__enter__()

            xt_sb = xt_pool.tile([128, DC, S], BF16)
            w_tiles = {}
            for wname in ("wq", "wk", "wv"):
                w_tiles[wname] = w_pool.tile(
                    [128, DC, GM], BF16, tag=wname, name=f"w_{wname}"
                )

            def _load_w(wname, wd):
                for dc in range(DC):
                    nc.sync.dma_start(
                        w_tiles[wname][:, dc, :], wd[128 * dc : 128 * (dc + 1), :]
                    )

            # Warmup source: the HAM clock gate keeps the PE at 1.2 GHz
            # until it has been ~3.4us busy. Garbage matmuls with no DMA
            # dependency flip it to 2.4 GHz while real operands stream in.
            # A raw (non-pool) SBUF tensor skips the tile framework's
            # written-before-read check, so no memset gates the first LDW.
            dummy_cm = nc.sbuf_tensor("warmup_src", [128, 512], BF16)
            dummy_sb = dummy_cm.__enter__()
            # DMA order matters: wq first, then xt chunk-by-chunk so the
            # first QT chain starts early and paces with chunk arrival
            # (each stall < HAM's 3.4us window), then wk/wv.
            # Pass 1 (wq-mtp0) is the only DMA-paced pass: it needs xt
            # plus just the FIRST half of each wq chunk (heads 0-1 =
            # columns 0:256). Defer the second halves to after the xt
            # stream so the bandwidth-limited window carries 9MB, not
            # 10MB, and chunk 1 issues two slots earlier.
            for dc in range(DC):
                if dc == 0:
                    # fine-grained first chunk: the very first matmul
                    # needs only wq[0][:, 0:128] + xt[0][:, 0:512]
                    nc.sync.dma_start(
                        w_tiles["wq"][:, 0, 0:128], wqt_d[0:128, 0:128]
                    )
                    nc.sync.dma_start(xt_sb[:, 0, 0:1024], xt_d[0:128, 0:1024])
                    nc.sync.dma_start(
                        w_tiles["wq"][:, 0, 128:256], wqt_d[0:128, 128:256]
                    )
                    nc.sync.dma_start(
                        xt_sb[:, 0, 1024:2048], xt_d[0:128, 1024:2048]
                    )
                    continue
                nc.sync.dma_start(
                    xt_sb[:, dc, :], xt_d[128 * dc : 128 * (dc + 1), :]
                )
                nc.sync.dma_start(
                    w_tiles["wq"][:, dc, 0:256], wqt_d[128 * dc : 128 * (dc + 1), 0:256]
                )
            for dc in range(DC):
                nc.sync.dma_start(
                    w_tiles["wq"][:, dc, 256:512],
                    wqt_d[128 * dc : 128 * (dc + 1), 256:512],
                )
            _load_w("wk", wkt_d)
            _load_w("wv", wvt_d)
            # Constants from host (needed only for attention — emitted after
            # the projection-critical DMAs): all-ones column for the
            # denominator matmuls, and 4 shifted-triangular causal masks
            # (variant r keeps q >= k + 128r).
            ones_sb = const_pool.tile([128, 1], BF16)
            nc.sync.dma_start(ones_sb[:], ones_d[:])
            masks_sb = const_pool.tile([128, NQB, 512], BF16)
            nc.sync.dma_start(
                masks_sb[:], masks_d[:].rearrange("p (r q) -> p r q", r=NQB)
            )

            # QT / KT: out tile [m=128, s=512], contraction over d.
            # dc outer over PAIRS of m tiles = 8 concurrent psum chains
            # (all 8 banks): per xt chunk the PE has ~1.7us of work, which
            # matches the chunk DMA arrival rate, so the in-order PE queue
            # does not stall during the load ramp. LDWEIGHTS amortized 4x
            # over the st4-minor matmuls.
            with tc.tile_pool(name="psum_qtkt", bufs=8, space="PSUM") as psum1:
                wu_ps = psum1.tile([128, 512], F32, tag="ps1", name="warmup_ps")
                for _ in range(16):
                    nc.tensor.matmul(
                        wu_ps[:], lhsT=dummy_sb[:, 0:128], rhs=dummy_sb[:],
                        start=True, stop=True,
                    )
                for wname, out_sb in (("wq", qt_sb), ("wk", kt_sb)):
                    w_sb = w_tiles[wname]
                    for mtp in range(NH // 2):
                        last = wname == "wk" and mtp == NH // 2 - 1
                        pss = [
                            psum1.tile(
                                [128, 512],
                                F32,
                                tag="ps1",
                                name=f"ps1_{wname}_{mtp}_{i}",
                            )
                            for i in range(8)
                        ]
                        def _copy_out(j, st4):
                            dst = out_sb[
                                :, 2 * mtp + j, 512 * st4 : 512 * (st4 + 1)
                            ]
                            if last and (2 * j + st4) % 2 == 0:
                                # split the final copy burst across ACT and
                                # DVE so the V projection (which reuses
                                # these psum banks) isn't gated on ~5us of
                                # serial DVE casts — that stall crossed the
                                # HAM window and re-throttled the PE clock.
                                nc.scalar.activation(
                                    dst,
                                    pss[4 * j + st4][:],
                                    mybir.ActivationFunctionType.Copy,
                                )
                            else:
                                nc.vector.tensor_copy(dst, pss[4 * j + st4][:])

                        for dc in range(DC):
                            for j in (0, 1):
                                mt = 2 * mtp + j
                                for st4 in range(NQB):
                                    nc.tensor.matmul(
                                        pss[4 * j + st4][:],
                                        lhsT=w_sb[:, dc, 128 * mt : 128 * (mt + 1)],
                                        rhs=xt_sb[
                                            :, dc, 512 * st4 : 512 * (st4 + 1)
                                        ],
                                        start=(dc == 0),
                                        stop=(dc == DC - 1),
                                    )
                                    if last and dc == DC - 1:
                                        # copy each chain right after its
                                        # stop-matmul, under the remaining
                                        # chains' matmul stream
                                        _copy_out(j, st4)
                        if not last:
                            for j in (0, 1):
                                for st4 in range(NQB):
                                    _copy_out(j, st4)

            # -------- attention psum pools (coexist with the V pool) -----
            # left stack: psum_o (banks 0-1), psum_v (2-3, exits after V);
            # right: psum_s (4-7, two [128,1024] tiles). After V exits:
            # psum_d (bank 2) and psum_3 (bank 3) take psum_v's banks.
            psum_o_cm = tc.tile_pool(name="psum_o", bufs=2, space="PSUM")
            psum_o = psum_o_cm.__enter__()
            psum_v_cm = tc.tile_pool(name="psum_v", bufs=2, space="PSUM")
            psum_v = psum_v_cm.__enter__()
            psum_s_cm = tc.tile_pool(name="psum_s", bufs=2, space="PSUM", side="right")
            psum_s = psum_s_cm.__enter__()

            # Pools bound later (phase C); closures below resolve the names
            # at call time.
            psum_d = psum_3 = small_pool = wot_sb = None

            # ---------------- Attention group machinery ----------------
            def make_score_ops(h, qb, exp_pool, dpair_pool, fixed=None):
                # One op per PAIR of k tiles: two scores matmuls into the
                # two banks of a [128, 1024] PSUM tile, one wide exp, then
                # the causal mask multiplies, then a DVE pair-sum that
                # halves the denominator matmul count.
                nkt = 4 * qb + 4  # k tiles with any unmasked element
                if fixed is None:
                    eT = exp_pool.tile(
                        [128, nkt, 512], BF16, tag="eT", name=f"eT_{h}_{qb}"
                    )
                    dp = dpair_pool.tile(
                        [128, nkt // 2, 512], BF16, tag="dpair", name=f"dp_{h}_{qb}"
                    )
                else:
                    eT = exp_pool.tile(
                        [128, fixed, 512], BF16, tag="eT", name=f"eT_{h}_{qb}"
                    )[:, :nkt, :]
                    dp = dpair_pool.tile(
                        [128, fixed // 2, 512], BF16, tag="dpair",
                        name=f"dp_{h}_{qb}",
                    )[:, : nkt // 2, :]

                def score_pair(kt0):
                    # Diagonal k tiles (r = kt - 4qb >= 0) only have valid
                    # scores for q >= 128r in this block: trim the matmul
                    # to that range. The exp still covers the full pair
                    # tile (the trimmed region holds garbage that is never
                    # read: O/D matmuls are trimmed identically, and the
                    # pair-sum strip is zeroed below).
                    ps = psum_s.tile(
                        [128, 1024], F32, tag="ps_s", name=f"ps_s_{h}_{qb}_{kt0}"
                    )
                    for j in (0, 1):
                        kt = kt0 + j
                        r = kt - 4 * qb
                        qo = 128 * r if r > 0 else 0
                        nc.tensor.matmul(
                            ps[:, 512 * j + qo : 512 * (j + 1)],
                            lhsT=kt_sb[:, h, 128 * kt : 128 * (kt + 1)],
                            rhs=qt_sb[:, h, 512 * qb + qo : 512 * (qb + 1)],
                            start=True,
                            stop=True,
                        )
                    r1 = kt0 + 1 - 4 * qb
                    if r1 < 0:
                        # non-diagonal pair: one wide exp over both banks
                        nc.scalar.activation(
                            eT[:, kt0 : kt0 + 2, :].rearrange("p a b -> p (a b)"),
                            ps[:],
                            mybir.ActivationFunctionType.Exp,
                            scale=SCALE,
                        )
                    else:
                        # diagonal pair: exact-range exps (the trimmed
                        # PSUM regions were never written)
                        for j in (0, 1):
                            r = kt0 + j - 4 * qb
                            qo = 128 * r if r > 0 else 0
                            nc.scalar.activation(
                                eT[:, kt0 + j, qo:512],
                                ps[:, 512 * j + qo : 512 * (j + 1)],
                                mybir.ActivationFunctionType.Exp,
                                scale=SCALE,
                            )
                    for j in (0, 1):
                        r = kt0 + j - 4 * qb
                        if r >= 0:
                            # only the 128-wide diagonal strip can have
                            # masked elements; beyond it the mask is all 1
                            qo = 128 * r
                            qe = qo + 128
                            nc.vector.tensor_mul(
                                eT[:, kt0 + j, qo:qe],
                                eT[:, kt0 + j, qo:qe],
                                masks_sb[:, r, qo:qe],
                            )
                    r1 = kt0 + 1 - 4 * qb
                    if r1 >= 1:
                        qo0 = 128 * (r1 - 1)
                        qo1 = 128 * r1
                        # kt0+1's exp starts at qo1; below it only kt0
                        # contributes — copy that strip instead of zeroing
                        # the garbage and paying for a wider add
                        nc.vector.tensor_copy(
                            dp[:, kt0 // 2, qo0:qo1], eT[:, kt0, qo0:qo1]
                        )
                        nc.vector.tensor_add(
                            dp[:, kt0 // 2, qo1:512],
                            eT[:, kt0, qo1:512],
                            eT[:, kt0 + 1, qo1:512],
                        )
                    else:
                        nc.vector.tensor_add(
                            dp[:, kt0 // 2, :], eT[:, kt0, :], eT[:, kt0 + 1, :]
                        )
                        # second-level sum for clean (non-diagonal) quads:
                        # dp[2q] += dp[2q+1] in place, halving the [1,512]
                        # denominator matmuls for the off-diagonal k range
                        p = kt0 // 2
                        if p % 2 == 1 and p <= 2 * qb - 1:
                            nc.vector.tensor_add(
                                dp[:, p - 1, :], dp[:, p - 1, :], dp[:, p, :]
                            )

                return (
                    eT,
                    dp,
                    [lambda k=k: score_pair(2 * k) for k in range(nkt // 2)],
                )

            def make_av_ops(h, qb, eT, dp):
                nkt = 4 * qb + 4
                qsl = slice(512 * qb, 512 * (qb + 1))
                state = {}

                def o_op(kt):
                    if kt == 0:
                        state["psO"] = psum_o.tile(
                            [128, 512], F32, tag="ps_o", name=f"psO_{h}_{qb}"
                        )
                    r = kt - 4 * qb
                    qo = 128 * r if r > 0 else 0
                    nc.tensor.matmul(
                        state["psO"][:, qo:512],
                        lhsT=v_sb[:, kt, 128 * h : 128 * (h + 1)],
                        rhs=eT[:, kt, qo:512],
                        start=(kt == 0),
                        stop=(kt == nkt - 1),
                    )

                clean = [2 * q for q in range(qb)]  # quad-summed reads
                n_d = len(clean) + 2

                def d_op(i):
                    # Over the DVE tree sums; emitted as one consecutive
                    # burst so the identical all-ones LDWEIGHTS dedupe
                    # down to a single load.
                    if i == 0:
                        state["psD"] = psum_d.tile(
                            [1, 512], F32, tag="ps_d", name=f"psD_{h}_{qb}"
                        )
                    if i < len(clean):
                        p, qo = clean[i], 0
                    else:
                        p = 2 * qb + (i - len(clean))  # diagonal pair
                        r0 = 2 * p - 4 * qb
                        qo = 128 * r0 if r0 > 0 else 0
                    nc.tensor.matmul(
                        state["psD"][0:1, qo:512],
                        lhsT=ones_sb[:, :],
                        rhs=dp[:, p, qo:512],
                        start=(i == 0),
                        stop=(i == n_d - 1),
                    )

                def fin_a():
                    # reciprocal_approx_fast (~18 bits; denominators are
                    # well in range) straight from PSUM, broadcast on
                    # GPSIMD. Emitted right after the D chain so the slow
                    # (~1-2us) broadcast overlaps the slot's scores/AV
                    # instead of gating the psum_o recycle at its end.
                    rcp = small_pool.tile(
                        [1, 512], F32, tag="rcp", name=f"rcp_{h}_{qb}"
                    )
                    nc.vector.reciprocal_approx_fast(rcp[:], state["psD"][:])
                    rcpb = small_pool.tile(
                        [128, 512], F32, tag="rcpb", name=f"rcpb_{h}_{qb}"
                    )
                    nc.gpsimd.partition_broadcast(rcpb[:], rcp[:])
                    state["rcpb"] = rcpb

                def fin_b():
                    # normalize on DVE once the AV chain has the psO ready
                    nc.vector.tensor_mul(
                        ao_sb[:, h, qsl], state["psO"][:], state["rcpb"][:]
                    )

                return (
                    [lambda kt=kt: o_op(kt) for kt in range(nkt)],
                    [lambda i=i: d_op(i) for i in range(n_d)],
                    fin_a,
                    fin_b,
                )

            # ----- V projection with round-0 scores interleaved -----
            # V: out tile [s=128, dv=512]; stationary is the xt chunk, so
            # every matmul reloads weights — the 64-deep PE queue pulls the
            # LDWEIGHTS ahead of the running matmul, hiding the reload.
            grp0 = {}
            for st in range(S // 128):
                ps = psum_v.tile([128, 512], F32, tag="psv", name=f"psv_{st}")
                w_sb = w_tiles["wv"]
                for dc in range(DC):
                    nc.tensor.matmul(
                        ps[:],
                        lhsT=xt_sb[:, dc, 128 * st : 128 * (st + 1)],
                        rhs=w_sb[:, dc, :],
                        start=(dc == 0),
                        stop=(dc == DC - 1),
                    )
                nc.vector.tensor_copy(v_sb[:, st, :], ps[:])
                # Prime the attention pipeline: scores for all of round 0
                # under the early V chains. QT/KT are long done; the exps
                # run on ACT underneath the remaining V stream.
                if st in (0, 1, 2, 4):
                    h = 3 if st == 4 else st
                    eT, dp, s_ops = make_score_ops(h, 0, exp0_pool, dp0_pool)
                    for op in s_ops:
                        op()
                    grp0[h] = (eT, dp)

            psum_v_cm.__exit__(None, None, None)
            psum_d_cm = tc.tile_pool(name="psum_d", bufs=1, space="PSUM")
            psum_d = psum_d_cm.__enter__()
            psum_3_cm = tc.tile_pool(name="psum_3", bufs=1, space="PSUM")
            psum_3 = psum_3_cm.__enter__()

            # xt / weights are dead; the attention + o_proj SBUF working
            # set replaces them (right-side frees don't disturb the left
            # stack the new pools go on).
            w_cm.__exit__(None, None, None)
            xt_cm.__exit__(None, None, None)

            exp_cm = tc.tile_pool(name="exp", bufs=2)
            exp_pool = exp_cm.__enter__()
            dpair_cm = tc.tile_pool(name="dpair", bufs=2)
            dpair_pool = dpair_cm.__enter__()
            small_cm = tc.tile_pool(name="small", bufs=3)
            small_pool = small_cm.__enter__()
            wot_cm = tc.tile_pool(name="wot", bufs=1)
            wot_pool = wot_cm.__enter__()
            out_cm = tc.tile_pool(name="ostage", bufs=6)
            out_pool = out_cm.__enter__()

            wot_sb = wot_pool.tile([128, NH, D], BF16)
            for c4 in range(NH):
                nc.sync.dma_start(
                    wot_sb[:, c4, :], wot_d[128 * c4 : 128 * (c4 + 1), :]
                )

            # ---------------- Fused attention + o_proj rounds ----------
            def p3_chain(st, nt):
                ps = psum_3.tile([128, 512], F32, tag="ps3", name=f"ps3_{st}_{nt}")
                for h in range(NH):
                    nc.tensor.matmul(
                        ps[:],
                        lhsT=ao_sb[:, h, 128 * st : 128 * (st + 1)],
                        rhs=wot_sb[:, h, 512 * nt : 512 * (nt + 1)],
                        start=(h == 0),
                        stop=(h == NH - 1),
                    )
                o_sb = out_pool.tile(
                    [128, 512], F32, tag="ost", name=f"ost_{st}_{nt}"
                )
                # copies on DVE: ACT's FIFO must stay clear for the exps
                # that pace the whole attention pipeline
                nc.vector.tensor_copy(o_sb[:], ps[:])
                nc.sync.dma_start(
                    out_d[128 * st : 128 * (st + 1), 512 * nt : 512 * (nt + 1)],
                    o_sb[:],
                )

            pend_p3 = []  # ready (st, nt) o_proj chains, FIFO

            # Two-slot software pipeline: slot i emits the scores of group
            # X[i] and the AV/denominator/normalize of group Y[i] = the
            # group scored two slots earlier. The 2-slot lead means the
            # exp (ACT) of a group is long done before its AV matmuls, so
            # the PE never stalls on the exp pipeline; the eT/dp rings
            # (bufs=2) self-throttle ACT/DVE against the PE's progress.
            # Round 0 was scored under the V stream, so Y starts at round
            # 0 while X starts at round 1.
            records = dict(grp0)  # (h,qb)->(eT,dp) — round 0 keyed by h
            x_seq = (
                [(0, 1), (1, 1), None, None, (2, 1), (3, 1)]
                + [(h, qb) for qb in (2, 3) for h in range(NH)]
                + [None, None]
            )
            y_seq = [(h, qb) for qb in range(NQB) for h in range(NH)]
            for X, Y in zip(x_seq, y_seq):
                yh, yqb = Y
                eT, dp = records.pop(yh if yqb == 0 else Y)
                o_ops, d_ops, fin_a, fin_b = make_av_ops(yh, yqb, eT, dp)
                n_o = len(o_ops)
                emitted = 0
                while emitted < min(2, n_o):
                    o_ops[emitted]()
                    emitted += 1
                # denominator chain + reciprocal/broadcast up front: the
                # dp sums are two slots old, and the broadcast latency
                # hides under this slot's scores/AV stream
                for op in d_ops:
                    op()
                fin_a()
                if X is not None:
                    eTx, dpx, s_ops = make_score_ops(
                        X[0], X[1], exp_pool, dpair_pool, fixed=16
                    )
                    records[X] = (eTx, dpx)
                    n_s = len(s_ops)
                    for i, s in enumerate(s_ops):
                        s()
                        want = (i + 1) * n_o // n_s
                        while emitted < want:
                            o_ops[emitted]()
                            emitted += 1
                        # one fused o_proj chain per score pair keeps
                        # ~0.9us of other PE work between chains (psum_3
                        # is one bank, so back-to-back chains would stall
                        # on the DVE drain).
                        if pend_p3 and i >= 1:
                            p3_chain(*pend_p3.pop(0))
                while emitted < n_o:
                    o_ops[emitted]()
                    emitted += 1
                    if X is None and pend_p3 and emitted % 2 == 0:
                        p3_chain(*pend_p3.pop(0))
                fin_b()
                if yh == NH - 1:
                    # round yqb fully normalized: its 16 o_proj chains are
                    # ready to fuse into the following slots' streams.
                    pend_p3.extend(
                        (4 * yqb + sti, nt)
                        for sti in range(4)
                        for nt in range(NQB)
                    )
            # drain any chains that didn't fit between the last score pairs
            # while the single-bank pool still exists; spacing no longer
            # matters once PSUM copies are the only outstanding PE deps...
            # instead, swap to a 4-bank pool for the trailing burst.
            psum_3_cm.__exit__(None, None, None)
            psum_d_cm.__exit__(None, None, None)
            psum_s_cm.__exit__(None, None, None)

            # ---------------- trailing o_proj chains ----------------
            # Whatever is left (at least round 3): 4 concurrent chains so
            # the psum->SBUF copies overlap the next chains' matmuls.
            with tc.tile_pool(name="psum_3t", bufs=4, space="PSUM") as psum_3t:
                rest = list(pend_p3)
                pend_p3.clear()
                by_st = {}
                for st, nt in rest:
                    by_st.setdefault(st, []).append(nt)
                for st, nts in by_st.items():
                    # nt-outer / h-inner: each chain completes after its 4
                    # matmuls, so its copy (alternating DVE/ACT) overlaps
                    # the remaining chains and the next st's ring reuse
                    # never waits on a copy still in flight.
                    for nt in nts:
                        ps = psum_3t.tile(
                            [128, 512], F32, tag="ps3t", name=f"ps3t_{st}_{nt}"
                        )
                        for h in range(NH):
                            nc.tensor.matmul(
                                ps[:],
                                lhsT=ao_sb[:, h, 128 * st : 128 * (st + 1)],
                                rhs=wot_sb[:, h, 512 * nt : 512 * (nt + 1)],
                                start=(h == 0),
                                stop=(h == NH - 1),
                            )
                        o_sb = out_pool.tile(
                            [128, 512], F32, tag="ost", name=f"ost_{st}_{nt}"
                        )
                        if nt % 2 == 0:
                            nc.scalar.activation(
                                o_sb[:],
                                ps[:],
                                mybir.ActivationFunctionType.Copy,
                            )
                        else:
                            nc.vector.tensor_copy(o_sb[:], ps[:])
                        nc.sync.dma_start(
                            out_d[
                                128 * st : 128 * (st + 1),
                                512 * nt : 512 * (nt + 1),
                            ],
                            o_sb[:],
                        )
            psum_o_cm.__exit__(None, None, None)
            out_cm.__exit__(None, None, None)
            wot_cm.__exit__(None, None, None)
            small_cm.__exit__(None, None, None)
            dpair_cm.__exit__(None, None, None)
            exp_cm.__exit__(None, None, None)
    _dedup_ldweights(nc)
    _split_excess_waits(nc)
    # Populate .instr bytes for extended-inst InstISA subclasses
    # (InstPartitionBroadcast) — raw Bass skips this Bacc pass and the NEFF
    # compiler errors with "ISA wrong length" without it.
    lower_extended_insts(nc)
    return nc


def _prep_in_maps(in_features, q_proj, k_proj, v_proj, o_proj):
    # Host-side prep in numpy — np.asarray first so jax-array inputs don't
    # route the transpose/cast through a device backend.
    in_features = np.asarray(in_features)
    q_proj = np.asarray(q_proj)
    k_proj = np.asarray(k_proj)
    v_proj = np.asarray(v_proj)
    o_proj = np.asarray(o_proj)
    bf = ml_dtypes.bfloat16
    # mask variant r: [128, 512] keeping (1.0) where q >= k + 128r, else 0.
    k_idx = np.arange(128)[:, None]
    q_idx = np.arange(512)[None, :]
    masks = np.concatenate(
        [(q_idx >= k_idx + 128 * r) for r in range(NQB)], axis=1
    ).astype(bf)
    ones = np.ones((128, 1), bf)
    in_maps = []
    for c in range(N_CORES):
        b, g = divmod(c, 4)
        ms = slice(512 * g, 512 * (g + 1))
        in_maps.append(
            {
                "xt": in_features[b].T.astype(bf),
                "wqt": q_proj[ms, :].T.astype(bf),
                "wkt": k_proj[ms, :].T.astype(bf),
                "wvt": v_proj[ms, :].T.astype(bf),
                "wot": o_proj[:, ms].T.astype(bf),
                "masks": masks,
                "ones": ones,
            }
        )
    return in_maps


def _run(inputs, trace=False):
    nc = build_bass()
    in_maps = _prep_in_maps(**inputs)
    res = run_bass_kernel_spmd(nc, in_maps, list(range(N_CORES)), trace=trace)
    B = inputs["in_features"].shape[0]
    out = np.zeros((B, S, D), np.float32)
    for c in range(N_CORES):
        out[c // 4] += res.results[c]["out"]
    return out, res


def kernel(**inputs):
    out, _ = _run(inputs, trace=False)
    return out
